# revision 1
# baseline (speedup 1.0000x reference)
"""Chamfer loss on 8 Trainium2 NeuronCores.

Reference: P[b,i,j] = ||gts[b,i]||^2 + ||preds[b,j]||^2 - 2<gts[b,i],preds[b,j]>
loss = sum_j min_i P  +  sum_i min_j P   (summed over batches)

Sharding: data-parallel over batch (16 batches -> 8 cores x 2).
Host marshals each batch into augmented matmul operands so that a single
PE matmul produces a full 128x512 tile of P directly:
  XA rows: [x0, x1, x2, rx, 1]      (gts,   rx = ||x||^2)
  YA rows: [-2*y0, -2*y1, -2*y2, 1, ry]  (preds, ry = ||y||^2)
  P = XA^T @ YA
bf16 matmul alone is too coarse (true mins are ~1e-3 while terms are O(10)),
so each operand is split hi/lo (XA = XAh + XAl with XAh = bf16(XA)) and the
three significant products are folded into ONE K=15 bf16 matmul:
  P ~= [XAh;XAh;XAl]^T @ [YAh;YAl;YAh]   (only the lo*lo term is dropped)

Device dataflow per batch (negated domain: rowbuf holds -P so all mins
become maxes, enabling the fast Max8 instruction; the total is negated
on-device at the end):
  PE:  K=15 bf16 matmul (full rate) -> PSUM [128 i, 512 j] tiles
  ACT: copy PSUM -> SBUF bf16 with scale=-1, FD=2048 chunks (row buffer)
  DVE: col-min: colacc = max(colacc, rowbuf)        [128,4096] bf16 2x mode
       row-min: nc.vector.max (Max8 top-8) -> take element 0
  PE:  transpose colacc 128x128 blocks -> PSUM; DVE reduce-max -> col-mins
  DVE: sum everything, negate -> per-partition partials [128,1] fp32 -> DRAM
Host sums the 8 cores' partials.
"""

import numpy as np

B, N, D = 16, 4096, 3
NCORES = 8
BPC = B // NCORES  # batches per core
IT = 128  # i-tile (PSUM partitions)
JT = 512  # j-tile (one PSUM bank of fp32)
NIT = N // IT  # 32
NJT = N // JT  # 8

_CACHE = {}


def _build(repeat: int = 1, mode: str = "full", tail: str = "transpose", rbufs: int = 3, fbufs: int = 2, rowmin_ttr: bool = False, dma_chunks: int = 1, pmm_bufs: int = 2, pt_bufs: int = 2, rowmin_direct: bool = False, rowmin_max8: bool = True, act2k: bool = True):
    from contextlib import ExitStack

    import concourse.bass as bass
    import concourse.tile as tile
    from concourse import bacc, mybir

    from concourse import bass_isa

    f32 = mybir.dt.float32
    bf16 = mybir.dt.bfloat16
    f32r = mybir.dt.float32r
    MIN = mybir.AluOpType.min
    ADD = mybir.AluOpType.add
    AX = mybir.AxisListType.X
    ts = bass.ts

    nc = bacc.Bacc(
        "TRN2", target_bir_lowering=False, debug=False, num_devices=NCORES
    )
    xa = nc.dram_tensor("xa", [BPC, 15, N], bf16, kind="ExternalInput").ap()
    ya = nc.dram_tensor("ya", [BPC, 15, N], bf16, kind="ExternalInput").ap()
    ident = nc.dram_tensor("ident", [128, 128], bf16, kind="ExternalInput").ap()
    out = nc.dram_tensor("out", [128, 1], f32, kind="ExternalOutput").ap()
    reps = nc.dram_tensor("reps", [128, 1], f32, kind="ExternalOutput").ap()

    with tile.TileContext(nc) as tc, ExitStack() as ctx:
        const_pool = ctx.enter_context(tc.tile_pool(name="const", bufs=1))
        rowbuf_pool = ctx.enter_context(tc.tile_pool(name="rowbuf", bufs=rbufs))
        fold_pool = ctx.enter_context(tc.tile_pool(name="fold", bufs=fbufs))
        acc_pool = ctx.enter_context(tc.tile_pool(name="acc", bufs=1))
        psum_pool = ctx.enter_context(
            tc.tile_pool(name="psum", bufs=pmm_bufs, space="PSUM")
        )
        psumT_pool = ctx.enter_context(
            tc.tile_pool(name="psumT", bufs=pt_bufs, space="PSUM")
        )

        id_sb = const_pool.tile([128, 128], bf16, tag="ident")
        nc.gpsimd.dma_start(id_sb[:], ident)

        XA, YA = [], []
        for b in range(BPC):
            xt = const_pool.tile([15, N], bf16, tag=f"xa{b}", name=f"xat{b}")
            yt = const_pool.tile([15, N], bf16, tag=f"ya{b}", name=f"yat{b}")
            if dma_chunks > 1:
                cw = N // dma_chunks
                for c in range(dma_chunks):
                    nc.gpsimd.dma_start(
                        xt[:, ts(c, cw)], xa[b, :, ts(c, cw)]
                    )
                    nc.gpsimd.dma_start(
                        yt[:, ts(c, cw)], ya[b, :, ts(c, cw)]
                    )
            else:
                nc.gpsimd.dma_start(xt[:], xa[b])
                nc.gpsimd.dma_start(yt[:], ya[b])
            XA.append(xt)
            YA.append(yt)

        colsum_neg = [
            acc_pool.tile([1, 1], f32, tag=f"csn{b}", name=f"csn{b}")
            for b in range(BPC)
        ]
        rep_cnt = acc_pool.tile([128, 1], f32, tag="rep_cnt")
        nc.vector.memset(rep_cnt[:], 0.0)

        rowmins = [
            acc_pool.tile([128, NIT], f32, tag=f"rm{b}", name=f"rm{b}") for b in range(BPC)
        ]
        colmins = [
            acc_pool.tile([128, NIT], f32, tag=f"cm{b}", name=f"cm{b}") for b in range(BPC)
        ]

        for b in [b for _ in range(repeat) for b in range(BPC)]:
            nc.vector.tensor_scalar_add(rep_cnt[:], rep_cnt[:], 1.0 / BPC)
            colacc = acc_pool.tile([128, N], bf16, tag=f"colacc{b}", name=f"colacc{b}")
            rowm8 = acc_pool.tile(
                [128, NIT * 8], bf16, tag=f"rowm8{b}", name=f"rowm8{b}"
            )
            for it in range(NIT):
                lhsT = XA[b][:, ts(it, IT)]
                rowbuf = rowbuf_pool.tile([128, N], bf16, tag="rowbuf")
                mmw = 4 if act2k else 2  # matmuls per PSUM tile
                for j2 in range(NJT // mmw):
                    ps = psum_pool.tile([128, mmw * JT], f32, tag="pmm")
                    for h in range(mmw):
                        jt = mmw * j2 + h
                        nc.tensor.matmul(
                            ps[:, ts(h, JT)],
                            lhsT,
                            YA[b][:, ts(jt, JT)],
                            start=True,
                            stop=True,
                        )
                    if mode != "mm":
                        if rowmin_max8:
                            # rowbuf holds -P so min-reductions become max
                            nc.scalar.mul(rowbuf[:, ts(j2, mmw * JT)], ps[:], -1.0)
                        else:
                            nc.scalar.copy(rowbuf[:, ts(j2, mmw * JT)], ps[:])
                if mode == "mm" or mode == "mm_act":
                    continue
                # col-min accumulate (elementwise across i-tiles); first
                # i-tile initializes via a 4x-mode copy instead of memset+min
                if it == 0:
                    nc.vector.tensor_copy(colacc[:], rowbuf[:])
                else:
                    nc.vector.tensor_tensor(
                        colacc[:], colacc[:], rowbuf[:],
                        op=(mybir.AluOpType.max if rowmin_max8 else MIN),
                    )
                # row-min: fused half-fold + min-reduce in one DVE op
                if rowmin_max8:
                    nc.vector.max(rowm8[:, ts(it, 8)], rowbuf[:])
                elif rowmin_direct:
                    nc.vector.tensor_reduce(
                        rowmins[b][:, ts(it, 1)], rowbuf[:], axis=AX, op=MIN
                    )
                elif rowmin_ttr:
                    f1 = fold_pool.tile([128, N // 2], bf16, tag="f1")
                    nc.vector.tensor_tensor_reduce(
                        out=f1[:],
                        in0=rowbuf[:, : N // 2],
                        in1=rowbuf[:, N // 2 :],
                        scale=1.0,
                        scalar=3.0e38,
                        op0=MIN,
                        op1=MIN,
                        accum_out=rowmins[b][:, ts(it, 1)],
                    )
                else:
                    f1 = fold_pool.tile([128, N // 2], bf16, tag="f1")
                    nc.vector.tensor_tensor(
                        f1[:], rowbuf[:, : N // 2], rowbuf[:, N // 2 :], op=MIN
                    )
                    f2 = fold_pool.tile([128, N // 4], bf16, tag="f2")
                    nc.vector.tensor_tensor(
                        f2[:], f1[:, : N // 4], f1[:, N // 4 :], op=MIN
                    )
                    f3 = fold_pool.tile([128, N // 8], bf16, tag="f3")
                    nc.vector.tensor_tensor(
                        f3[:], f2[:, : N // 8], f2[:, N // 8 :], op=MIN
                    )
                    f4 = fold_pool.tile([128, N // 16], bf16, tag="f4")
                    nc.vector.tensor_tensor(
                        f4[:], f3[:, : N // 16], f3[:, N // 16 :], op=MIN
                    )
                    nc.vector.tensor_reduce(
                        rowmins[b][:, ts(it, 1)], f4[:], axis=AX, op=MIN
                    )
            if mode == "full" and rowmin_max8:
                nc.vector.tensor_copy(
                    rowmins[b][:, 0:NIT],
                    rowm8[:].rearrange("p (a b) -> p a b", b=8)[:, :, 0:1],
                )
            # batch tail: min over the partition axis of colacc.
            if mode == "full" and tail == "transpose":
                for blk in range(N // 128):
                    if act2k:
                        pst = psum_pool.tile(
                            [128, 128], bf16, tag="pmm", name="pstT"
                        )
                    else:
                        pst = psumT_pool.tile([128, 128], bf16, tag="pT")
                    nc.tensor.transpose(
                        pst[:], colacc[:, ts(blk, 128)], id_sb[:]
                    )
                    nc.vector.tensor_reduce(
                        colmins[b][:, ts(blk, 1)], pst[:], axis=AX,
                        op=(mybir.AluOpType.max if rowmin_max8 else MIN),
                    )
            elif mode == "full" and tail == "gpsimd":
                # negate (DVE 4x single-src) and max-all-reduce on GpSimd
                negacc = fold_pool.tile([128, N], bf16, tag="negacc")
                nc.vector.tensor_scalar_mul(negacc[:], colacc[:], -1.0)
                ar = acc_pool.tile(
                    [128, N], bf16, tag=f"ar{b}", name=f"ar{b}"
                )
                nc.gpsimd.partition_all_reduce(
                    ar[:], negacc[:], channels=128,
                    reduce_op=bass_isa.ReduceOp.max,
                )
                # colmin_j = -ar[0, j]; accumulate the negated sum
                nc.vector.tensor_reduce(
                    colsum_neg[b][:], ar[0:1, :], axis=AX, op=ADD
                )

        sums = acc_pool.tile([128, 2 * BPC * 2], f32, tag="sums")
        red_list = rowmins + (colmins if tail == "transpose" else [])
        if mode != "full":
            for t in rowmins + colmins:
                nc.vector.memset(t[:], 0.0)
            red_list = rowmins + colmins
        k = 0
        for t in red_list:
            nc.vector.tensor_reduce(sums[:, ts(k, 1)], t[:], axis=AX, op=ADD)
            k += 1
        # pad unused columns with zero so the final reduce is clean
        if k < sums.shape[1]:
            nc.vector.memset(sums[:, bass.ds(k, sums.shape[1] - k)], 0.0)
        total = acc_pool.tile([128, 1], f32, tag="total")
        nc.vector.tensor_reduce(total[:], sums[:], axis=AX, op=ADD)
        if rowmin_max8 and mode == "full":
            nc.vector.tensor_scalar_mul(total[:], total[:], -1.0)
        if mode == "full" and tail == "gpsimd":
            # total[0] -= sum_b colsum_neg[b]  (negated col sums)
            for b in range(BPC):
                nc.vector.tensor_tensor(
                    total[0:1, :], total[0:1, :], colsum_neg[b][:],
                    op=mybir.AluOpType.subtract,
                )
        nc.gpsimd.dma_start(out, total[:])
        nc.gpsimd.dma_start(reps, rep_cnt[:])

    nc.compile()
    return nc


def _get_nc(repeat: int = 1, mode: str = "full", tail: str = "transpose",
            rbufs: int = 3, fbufs: int = 2, rowmin_ttr: bool = False, dma_chunks: int = 1, pmm_bufs: int = 2, pt_bufs: int = 2, rowmin_direct: bool = False, rowmin_max8: bool = True, act2k: bool = True):
    key = f"nc{repeat}:{mode}:{tail}:{rbufs}:{fbufs}:{rowmin_ttr}"
    if key not in _CACHE:
        _CACHE[key] = _build(repeat, mode, tail, rbufs, fbufs, rowmin_ttr)
    return _CACHE[key]


def _prep_inputs(preds: np.ndarray, gts: np.ndarray):
    import ml_dtypes

    preds = np.ascontiguousarray(np.asarray(preds, dtype=np.float32))
    gts = np.ascontiguousarray(np.asarray(gts, dtype=np.float32))
    assert preds.shape == (B, N, D) and gts.shape == (B, N, D)

    bf = ml_dtypes.bfloat16
    ident = np.eye(128, dtype=bf)

    def hilo(a32):
        hi = a32.astype(bf)
        lo = (a32 - hi.astype(np.float32)).astype(bf)
        return hi, lo

    in_maps = []
    for c in range(NCORES):
        xs = np.empty((BPC, 15, N), dtype=bf)
        ys = np.empty((BPC, 15, N), dtype=bf)
        for bb in range(BPC):
            b = c * BPC + bb
            x = gts[b]  # [N, 3]
            y = preds[b]
            xa = np.empty((5, N), dtype=np.float32)
            ya = np.empty((5, N), dtype=np.float32)
            xa[0:3] = x.T
            xa[3] = (x.astype(np.float64) ** 2).sum(axis=1)
            xa[4] = 1.0
            ya[0:3] = -2.0 * y.T
            ya[3] = 1.0
            ya[4] = (y.astype(np.float64) ** 2).sum(axis=1)
            xh, xl = hilo(xa)
            yh, yl = hilo(ya)
            xs[bb] = np.concatenate([xh, xh, xl], axis=0)
            ys[bb] = np.concatenate([yh, yl, yh], axis=0)
        in_maps.append({"xa": xs, "ya": ys, "ident": ident})
    return in_maps


def _run(preds: np.ndarray, gts: np.ndarray, trace: bool = False):
    from concourse.bass_utils import run_bass_kernel_spmd

    nc = _get_nc()
    in_maps = _prep_inputs(preds, gts)
    res = run_bass_kernel_spmd(
        nc, in_maps, core_ids=list(range(NCORES)), trace=trace
    )
    partials = [res.results[c]["out"] for c in range(NCORES)]
    loss = np.float32(np.sum([p.astype(np.float64).sum() for p in partials]))
    return loss, res


def kernel(preds: np.ndarray, gts: np.ndarray) -> np.ndarray:
    loss, _ = _run(preds, gts, trace=False)
    return np.asarray(loss, dtype=np.float32)



# revision 14
# speedup vs baseline: 2.7544x; 2.7544x over previous
"""Chamfer loss on 8 Trainium2 NeuronCores.

Reference: P[b,i,j] = ||gts[b,i]||^2 + ||preds[b,j]||^2 - 2<gts[b,i],preds[b,j]>
loss = sum_j min_i P  +  sum_i min_j P   (summed over batches)

Sharding: data-parallel over batch (16 batches -> 8 cores x 2).
Host marshals each batch into augmented matmul operands so that a single
PE matmul produces a full 128x512 tile of P directly:
  XA rows: [x0, x1, x2, rx, 1]      (gts,   rx = ||x||^2)
  YA rows: [-2*y0, -2*y1, -2*y2, 1, ry]  (preds, ry = ||y||^2)
  P = XA^T @ YA
bf16 matmul alone is too coarse (true mins are ~1e-3 while terms are O(10)),
so each operand is split hi/lo (XA = XAh + XAl with XAh = bf16(XA)) and the
three significant products are folded into ONE K=15 bf16 matmul:
  P ~= [XAh;XAh;XAl]^T @ [YAh;YAl;YAh]   (only the lo*lo term is dropped)

Device dataflow per batch (negated domain: rowbuf holds -P so all mins
become maxes; the total is negated on-device at the end). Engine split per
i-tile (128 rows of P x 4096 cols, two 2048-wide PSUM groups):
  PE:  K=15 bf16 matmuls -> PSUM fp32 groups
  ACT: negate-copy PSUM group -> SBUF bf16 rowbuf           (~1.9us/group)
  DVE: col-min: colacc[j] = max(colacc[j], rowbuf[j]) (TT bf16, 2x mode)
       row-min: TTR(fold group halves, max-reduce) -> rmg[g][:, it]
       (TTR reads 2 operands/cycle: 2x the rate of Max8/TensorReduce)
  Pool (gpsimd): col-min for the last pool_w cols of each group, offloading
       the DVE.
Batch tail: PE transposes colacc 128x128 blocks (4 per PSUM tile); DVE
reduce-max over [128,4,128] -> col-mins; sums + negate -> out[128,1] fp32.
Host sums the 8 cores' partials.
"""

import numpy as np

B, N, D = 16, 4096, 3
NCORES = 8
BPC = B // NCORES  # batches per core
IT = 128  # i-tile (PSUM partitions)
JT = 512  # j-tile (one PSUM bank of fp32)
GW = 2048  # j-group width (ACT copy / DVE op granularity); 4 PSUM banks
NIT = N // IT  # 32
NG = N // GW  # groups per i-tile row (2)

_CACHE = {}


def _build(
    repeat: int = 1,
    mode: str = "full",
    rowmin: str = "fold2",  # "fold2" | "ttr" | "max8"
    pool_w: int = 0,  # j-width per group of col-acc offloaded to gpsimd
    rbufs: int = 3,
    fbufs: int = 2,
    pmm_bufs: int = 2,
    tail: str = "t4",  # "t4" | "t1" | "pool"
):
    from contextlib import ExitStack

    import concourse.bass as bass
    import concourse.tile as tile
    from concourse import bacc, bass_isa, mybir

    f32 = mybir.dt.float32
    bf16 = mybir.dt.bfloat16
    MAX = mybir.AluOpType.max
    ADD = mybir.AluOpType.add
    AX = mybir.AxisListType.X
    ts = bass.ts
    ds = bass.ds

    nc = bacc.Bacc(
        "TRN2", target_bir_lowering=False, debug=False, num_devices=NCORES
    )
    xa = nc.dram_tensor("xa", [BPC, 15, N], bf16, kind="ExternalInput").ap()
    ya = nc.dram_tensor("ya", [BPC, 15, N], bf16, kind="ExternalInput").ap()
    ident = nc.dram_tensor("ident", [128, 128], bf16, kind="ExternalInput").ap()
    out = nc.dram_tensor("out", [128, 1], f32, kind="ExternalOutput").ap()
    reps = nc.dram_tensor("reps", [128, 1], f32, kind="ExternalOutput").ap()

    dve_w = GW - pool_w  # leading cols of each group: col-acc on DVE

    with tile.TileContext(nc) as tc, ExitStack() as ctx:
        const_pool = ctx.enter_context(tc.tile_pool(name="const", bufs=1))
        rowbuf_pool = ctx.enter_context(tc.tile_pool(name="rowbuf", bufs=rbufs))
        fold_pool = ctx.enter_context(tc.tile_pool(name="fold", bufs=fbufs))
        acc_pool = ctx.enter_context(tc.tile_pool(name="acc", bufs=1))
        psum_pool = ctx.enter_context(
            tc.tile_pool(name="psum", bufs=pmm_bufs, space="PSUM")
        )

        id_sb = const_pool.tile([128, 128], bf16, tag="ident")
        nc.gpsimd.dma_start(id_sb[:], ident)

        XA, YA = [], []
        for b in range(BPC):
            xt = const_pool.tile([15, N], bf16, tag=f"xa{b}", name=f"xat{b}")
            yt = const_pool.tile([15, N], bf16, tag=f"ya{b}", name=f"yat{b}")
            nc.gpsimd.dma_start(xt[:], xa[b])
            nc.gpsimd.dma_start(yt[:], ya[b])
            XA.append(xt)
            YA.append(yt)

        rep_cnt = acc_pool.tile([128, 1], f32, tag="rep_cnt")
        nc.vector.memset(rep_cnt[:], 0.0)
        neginf = acc_pool.tile([128, 1], f32, tag="neginf")
        nc.vector.memset(neginf[:], -3.0e38)

        rowmins = [
            acc_pool.tile([128, NIT], f32, tag=f"rm{b}", name=f"rm{b}")
            for b in range(BPC)
        ]
        colmins = [
            acc_pool.tile([128, NIT], f32, tag=f"cm{b}", name=f"cm{b}")
            for b in range(BPC)
        ]
        # per-group row-min accumulators, combined into rowmins[b] per batch
        rmg = [
            [
                acc_pool.tile(
                    [128, NIT], f32, tag=f"rmg{b}_{g}", name=f"rmg{b}_{g}"
                )
                for g in range(NG)
            ]
            for b in range(BPC)
        ]

        for b in [b for _ in range(repeat) for b in range(BPC)]:
            nc.vector.tensor_scalar_add(rep_cnt[:], rep_cnt[:], 1.0 / BPC)
            colacc = acc_pool.tile(
                [128, N], bf16, tag=f"colacc{b}", name=f"colacc{b}"
            )
            if pool_w:
                for g in range(NG):
                    nc.gpsimd.memset(
                        colacc[:, ds(g * GW + dve_w, pool_w)], -3.0e38
                    )
            rowm8 = (
                acc_pool.tile([128, NIT * 8], bf16, tag=f"rowm8{b}", name=f"rowm8{b}")
                if rowmin in ("max8", "fold2")
                else None
            )
            for it in range(NIT):
                lhsT = XA[b][:, ts(it, IT)]
                rowbuf = rowbuf_pool.tile([128, N], bf16, tag="rowbuf")
                for g in range(NG):
                    ps = psum_pool.tile([128, GW], f32, tag="pmm")
                    for h in range(GW // JT):
                        jt = (GW // JT) * g + h
                        nc.tensor.matmul(
                            ps[:, ts(h, JT)],
                            lhsT,
                            YA[b][:, ts(jt, JT)],
                            start=True,
                            stop=True,
                        )
                    if mode == "mm":
                        continue
                    # ACT: negate-copy PSUM group -> SBUF bf16
                    nc.scalar.mul(rowbuf[:, ts(g, GW)], ps[:], -1.0)
                    if mode == "mm_act":
                        continue
                    # col-min accumulate, split DVE | Pool by column range
                    g0 = g * GW
                    if it == 0:
                        nc.vector.tensor_copy(
                            colacc[:, ds(g0, dve_w)], rowbuf[:, ds(g0, dve_w)]
                        )
                    else:
                        nc.vector.tensor_tensor(
                            colacc[:, ds(g0, dve_w)],
                            colacc[:, ds(g0, dve_w)],
                            rowbuf[:, ds(g0, dve_w)],
                            op=MAX,
                        )
                    if pool_w:
                        nc.gpsimd.tensor_tensor(
                            colacc[:, ds(g0 + dve_w, pool_w)],
                            colacc[:, ds(g0 + dve_w, pool_w)],
                            rowbuf[:, ds(g0 + dve_w, pool_w)],
                            op=MAX,
                        )
                    # row-min: fold group halves + max-reduce in one TTR
                    if rowmin == "ttr":
                        f1 = fold_pool.tile([128, GW // 2], bf16, tag="f1")
                        nc.vector.tensor_tensor_reduce(
                            out=f1[:],
                            in0=rowbuf[:, ds(g0, GW // 2)],
                            in1=rowbuf[:, ds(g0 + GW // 2, GW // 2)],
                            scale=1.0,
                            scalar=neginf[:],
                            op0=MAX,
                            op1=MAX,
                            accum_out=rmg[b][g][:, ts(it, 1)],
                        )
                if mode in ("mm", "mm_act"):
                    continue
                if rowmin == "max8":
                    nc.vector.max(rowm8[:, ts(it, 8)], rowbuf[:])
                elif rowmin == "fold2":
                    # fold 4096 -> 512 with TT max (2x mode), then Max8
                    f1 = fold_pool.tile([128, N // 2], bf16, tag="f1")
                    nc.vector.tensor_tensor(
                        f1[:], rowbuf[:, : N // 2], rowbuf[:, N // 2 :], op=MAX
                    )
                    f2 = fold_pool.tile([128, N // 4], bf16, tag="f2")
                    nc.vector.tensor_tensor(
                        f2[:], f1[:, : N // 4], f1[:, N // 4 :], op=MAX
                    )
                    f3 = fold_pool.tile([128, N // 8], bf16, tag="f3")
                    nc.vector.tensor_tensor(
                        f3[:], f2[:, : N // 8], f2[:, N // 8 :], op=MAX
                    )
                    nc.vector.max(rowm8[:, ts(it, 8)], f3[:])
            if mode in ("mm", "mm_act"):
                continue
            # combine per-i-tile row maxima into rowmins
            if rowmin == "ttr":
                nc.vector.tensor_tensor(
                    rowmins[b][:], rmg[b][0][:], rmg[b][1][:], op=MAX
                )
            else:
                nc.vector.tensor_copy(
                    rowmins[b][:, 0:NIT],
                    rowm8[:].rearrange("p (a b) -> p a b", b=8)[:, :, 0:1],
                )
            # batch tail: min over the partition axis of colacc
            if tail == "pool":
                ar = acc_pool.tile([128, N], bf16, tag=f"ar{b}", name=f"ar{b}")
                nc.gpsimd.partition_all_reduce(
                    ar[:], colacc[:], channels=128,
                    reduce_op=__import__("concourse.bass_isa", fromlist=["x"]).ReduceOp.max,
                )
                nc.vector.tensor_reduce(
                    colmins[b][:, 0:1], ar[0:1, :], axis=AX, op=MAX
                )
            elif tail == "t4":
                for q in range(N // 512):
                    pst = psum_pool.tile([128, 512], bf16, tag="pmm", name="pstT")
                    for k in range(4):
                        nc.tensor.transpose(
                            pst[:, ts(k, 128)],
                            colacc[:, ts(4 * q + k, 128)],
                            id_sb[:],
                        )
                    nc.vector.tensor_reduce(
                        colmins[b][:, ts(q, 4)],
                        pst[:].rearrange("p (a b) -> p a b", b=128),
                        axis=AX,
                        op=MAX,
                    )
            else:
                for blk in range(N // 128):
                    pst = psum_pool.tile([128, 128], bf16, tag="pmm", name="pstT")
                    nc.tensor.transpose(
                        pst[:], colacc[:, ts(blk, 128)], id_sb[:]
                    )
                    nc.vector.tensor_reduce(
                        colmins[b][:, ts(blk, 1)], pst[:], axis=AX, op=MAX
                    )

        sums = acc_pool.tile([128, 8], f32, tag="sums")
        red_list = rowmins + colmins
        if mode != "full":
            for t in rowmins + colmins:
                nc.vector.memset(t[:], 0.0)
        k = 0
        for t in red_list:
            nc.vector.tensor_reduce(sums[:, ts(k, 1)], t[:], axis=AX, op=ADD)
            k += 1
        if k < sums.shape[1]:
            nc.vector.memset(sums[:, ds(k, sums.shape[1] - k)], 0.0)
        total = acc_pool.tile([128, 1], f32, tag="total")
        # everything was accumulated in the negated domain
        nc.vector.tensor_reduce(total[:], sums[:], axis=AX, op=ADD, negate=True)
        nc.gpsimd.dma_start(out, total[:])
        nc.gpsimd.dma_start(reps, rep_cnt[:])

    nc.compile()
    return nc


def _get_nc(repeat: int = 1, **kw):
    key = (repeat, tuple(sorted(kw.items())))
    if key not in _CACHE:
        _CACHE[key] = _build(repeat, **kw)
    return _CACHE[key]


def _prep_inputs(preds: np.ndarray, gts: np.ndarray):
    import ml_dtypes

    preds = np.ascontiguousarray(np.asarray(preds, dtype=np.float32))
    gts = np.ascontiguousarray(np.asarray(gts, dtype=np.float32))
    assert preds.shape == (B, N, D) and gts.shape == (B, N, D)

    bf = ml_dtypes.bfloat16
    ident = np.eye(128, dtype=bf)

    def hilo(a32):
        hi = a32.astype(bf)
        lo = (a32 - hi.astype(np.float32)).astype(bf)
        return hi, lo

    in_maps = []
    for c in range(NCORES):
        xs = np.empty((BPC, 15, N), dtype=bf)
        ys = np.empty((BPC, 15, N), dtype=bf)
        for bb in range(BPC):
            b = c * BPC + bb
            x = gts[b]  # [N, 3]
            y = preds[b]
            xa = np.empty((5, N), dtype=np.float32)
            ya = np.empty((5, N), dtype=np.float32)
            xa[0:3] = x.T
            xa[3] = (x.astype(np.float64) ** 2).sum(axis=1)
            xa[4] = 1.0
            ya[0:3] = -2.0 * y.T
            ya[3] = 1.0
            ya[4] = (y.astype(np.float64) ** 2).sum(axis=1)
            xh, xl = hilo(xa)
            yh, yl = hilo(ya)
            xs[bb] = np.concatenate([xh, xh, xl], axis=0)
            ys[bb] = np.concatenate([yh, yl, yh], axis=0)
        in_maps.append({"xa": xs, "ya": ys, "ident": ident})
    return in_maps


def _run(preds: np.ndarray, gts: np.ndarray, trace: bool = False):
    from concourse.bass_utils import run_bass_kernel_spmd

    nc = _get_nc()
    in_maps = _prep_inputs(preds, gts)
    res = run_bass_kernel_spmd(
        nc, in_maps, core_ids=list(range(NCORES)), trace=trace
    )
    partials = [res.results[c]["out"] for c in range(NCORES)]
    loss = np.float32(np.sum([p.astype(np.float64).sum() for p in partials]))
    return loss, res


def kernel(preds: np.ndarray, gts: np.ndarray) -> np.ndarray:
    loss, _ = _run(preds, gts, trace=False)
    return np.asarray(loss, dtype=np.float32)
